# revision 1
# baseline (speedup 1.0000x reference)
"""Spatial-reduction attention (PVT-style) on 8 Trainium2 NeuronCores.

Sharding: core = (batch b, token half). Each core computes the full attention
output for its 2048 query tokens of its batch; the SR conv + LN + KV path
(1024 reduced tokens) is replicated across the 2 cores of a batch, so the
kernel needs no cross-core communication.

Structure: the attention phase (scores+softmax+AV) is paced by the Activation
engine (softmax exp), leaving TensorE ~30% idle there, while conv/KV/Q are
pure TensorE work. The loop body is therefore software-pipelined: stage A+B
(conv+LN+KV+Q) for iteration i+1 is emitted in small chunks interleaved into
iteration i's attention phase, with ping-pong kT/vaug/qT buffers. Chunk
drain pacing delays compute chunks until their input DMAs have landed, since
a premature matmul would head-of-line-block the in-order TensorE queue.

Score matmuls run in fp8e4m3 with DoubleRow perf mode (2 k-tiles per
instruction at 0.5 cycles/column): plane 0 pairs K with Q_hi = fp8(Q),
plane 1 pairs a second K copy with the residual Q - Q_hi, so the pair
computes K*(Q_hi + Q_res) ~= K*Q with only K's fp8 error left. Softmax
denominators are computed on a [128,16]-packed layout (cheap Ln/Exp) instead
of partition-starved [1,2048] rows.
"""
import numpy as np
import ml_dtypes

import concourse.bass as bass
import concourse.tile as tile
from concourse import mybir
from concourse.bass_utils import run_bass_kernel_spmd

import bass_rust

F32 = mybir.dt.float32
BF16 = mybir.dt.bfloat16
FP8 = mybir.dt.float8e4
DR = mybir.MatmulPerfMode.DoubleRow

B, N, C = 4, 4096, 512
H, HD = 8, 64
SIDE = 64           # sqrt(N)
RS = 32             # reduced side
NP = RS * RS        # 1024 reduced tokens
HALF = N // 2       # 2048 query tokens per core
LN_EPS = 1e-5
SCALE = HD ** -0.5

# weight order: "wtap" holds the 4 conv taps (.T); "wrest" holds wkT, wvT,
# wqT, wpT
W_K, W_V, W_Q, W_P = 0, 1, 2, 3


# --------------------------------------------------------------------------
# walrus workaround: this container's neuronx-cc rejects >1 sync-wait per
# instruction; hoist extras onto same-engine NoOps (program order preserved).
def _fixup_sync_waits(nc):
    fixed = 0
    for fn in nc.m.functions:
        for bb in fn.blocks:
            out = []
            changed = False
            for inst in bb.instructions:
                si = getattr(inst, "sync_info", None)
                waits = list(si.on_wait) if (si and si.on_wait) else []
                if len(waits) > 1:
                    for w in waits[:-1]:
                        nop = mybir.InstNoOp(
                            name=f"I-waitfix-{nc.next_id()}", ins=[], outs=[])
                        nop.engine = inst.engine
                        nop.sync_info = bass_rust.SyncInfo(
                            on_wait=[w], on_update=[])
                        out.append(nop)
                    si.on_wait = waits[-1:]
                    fixed += len(waits) - 1
                    changed = True
                out.append(inst)
            if changed:
                bb.instructions = out
    return fixed


# --------------------------------------------------------------------------
def build_nc(fixup=True, reps=1, trace_sim=False, loop_reps=None,
             use_bias=True, interleave=True, use_dr=True, parts="all"):
    nc = bass.Bass()
    dp = nc.declare_dram_parameter

    xtap_e = dp("xtap", [4 * C, NP], BF16, isOutput=False)  # conv tap gathers
    xth_e = dp("xth", [C, HALF], BF16, isOutput=False)      # own half of x.T
    wtap_e = dp("wtap", [4 * C, C], BF16, isOutput=False)   # sr_w taps .T
    wrest_e = dp("wrest", [4 * C, C], BF16, isOutput=False)  # wkT wvT wqT wpT
    srb_e = dp("srb", [1, C], BF16, isOutput=False)
    bk_e = dp("bkrow", [1, C], BF16, isOutput=False)
    bv_e = dp("bvrow", [1, C], BF16, isOutput=False)
    bp_e = dp("bp", [C, 1], F32, isOutput=False)
    id_e = dp("ident", [128, 128], BF16, isOutput=False)
    yt_e = dp("yT", [C, HALF], F32, isOutput=True)

    xtap_r = xtap_e.rearrange("(t p) n -> p t n", p=128)
    wtap_r = wtap_e.rearrange("(w p) n -> p w n", p=128)
    xth_r = xth_e.rearrange("(c p) n -> p c n", p=128)

    with tile.TileContext(nc, trace_sim=trace_sim) as tc:
        with tc.tile_pool(name="pp", bufs=1) as pp, \
             tc.tile_pool(name="ps", bufs=1, space="PSUM") as ps, \
             tc.tile_pool(name="dr", bufs=3, space="DRAM") as drp:
            wc = pp.tile([128, 16, C], BF16)
            nc.gpsimd.dma_start(out=wc,
                                in_=wrest_e.rearrange("(w p) n -> p w n",
                                                      p=128))
            ident = pp.tile([128, 128], BF16)
            nc.sync.dma_start(out=ident, in_=id_e[:])
            ones1 = pp.tile([1, 128], BF16)
            nc.vector.memset(ones1, 1.0)
            onesN = pp.tile([1, C], BF16)
            nc.vector.memset(onesN, 1.0)
            eps_t = pp.tile([128, 1], F32)
            nc.vector.memset(eps_t, LN_EPS)
            srb = pp.tile([1, C], BF16)
            nc.sync.dma_start(out=srb, in_=srb_e[:])
            bkrow = pp.tile([1, C], BF16)
            nc.sync.dma_start(out=bkrow, in_=bk_e[:])
            bvrow = pp.tile([1, C], BF16)
            nc.sync.dma_start(out=bvrow, in_=bv_e[:])
            bpt = pp.tile([128, 4], F32)
            nc.sync.dma_start(
                out=bpt, in_=bp_e.rearrange("(m p) o -> p (m o)", p=128))
            bp = [bpt[:, m:m + 1] for m in range(4)]

            def wslice(w, cc):
                return wc[:, w * 4 + cc, :]

            # cross-iteration ping-pong buffers
            kTz = [pp.tile([128, 4, 2, NP], FP8, tag=f"kT{b}", name=f"kT{b}")
                   for b in range(2)]
            vaug = [pp.tile([128, 8, H, HD + 1], BF16, tag=f"va{b}",
                            name=f"va{b}") for b in range(2)]
            qTb = [[pp.tile([128, 2, HALF], FP8, tag=f"qT{m}_{b}",
                            name=f"qT{m}_{b}") for m in range(4)]
                   for b in range(2)]
            for b in range(2):
                nc.vector.memset(vaug[b][:, :, :, HD:HD + 1], 1.0)

            # single-buffered staging (halved: reloaded twice per iteration)
            xTap = pp.tile([128, 16, 256], BF16, tag="xTap", name="xTap")
            wtap = pp.tile([128, 16, C], BF16, tag="wtap", name="wtap")
            xTh = pp.tile([128, 4, 1024], BF16, tag="xTh", name="xTh")
            xcT_a = pp.tile([128, 4, NP], BF16, tag="xcT", name="xcT")
            xcT = [xcT_a[:, cc] for cc in range(4)]
            outA = [pp.tile([128, HALF], BF16, tag=f"oA{m}", name=f"oA{m}")
                    for m in range(4)]

            # ------------- stage A+B emission chunks -------------
            def ab_chunks(dst):
                kT_, va_, qT_ = kTz[dst], vaug[dst], qTb[dst]

                def dma_xtap(quarter):
                    def go():
                        nc.sync.dma_start(
                            out=xTap,
                            in_=xtap_r[:, :, quarter * 256:
                                       (quarter + 1) * 256])
                    return go

                def dma_wtap():
                    def go():
                        for wch in range(4):
                            nc.sync.dma_start(
                                out=wtap[:, wch * 4:(wch + 1) * 4, :],
                                in_=wtap_r[:, wch * 4:(wch + 1) * 4, :])
                    return go

                def dma_xth(half):
                    def go():
                        nc.sync.dma_start(
                            out=xTh,
                            in_=xth_r[:, :, half * 1024:(half + 1) * 1024])
                    return go

                def conv_ct(ct):
                    def go():
                        col = (ct % 2) * 128
                        pc = ps.tile([128, C], F32, tag="mm", name="conv",
                                     bufs=2)
                        first = True
                        for t in range(4):
                            for cc in range(4):
                                nc.tensor.matmul(
                                    pc,
                                    xTap[:, t * 4 + cc, col:col + 128],
                                    wtap[:, t * 4 + cc, :],
                                    start=first,
                                    stop=(not use_bias and t == 3
                                          and cc == 3))
                                first = False
                        if use_bias:
                            nc.tensor.matmul(pc, ones1, srb,
                                             start=False, stop=True)
                        lnw = pp.tile([128, 10], F32, tag="lnw", name="lnw",
                                      bufs=3)
                        stats = lnw[:, 0:6]
                        mv = lnw[:, 6:8]
                        sd = lnw[:, 8:9]
                        rstd = lnw[:, 9:10]
                        xcF = pp.tile([128, C], BF16, tag="xcF", name="xcF",
                                      bufs=2)
                        nc.vector.tensor_copy(xcF, pc)
                        nc.vector.bn_stats(out=stats, in_=xcF)
                        nc.vector.bn_aggr(out=mv, in_=stats)
                        nc.scalar.activation(
                            out=sd, in_=mv[:, 1:2],
                            func=mybir.ActivationFunctionType.Ln,
                            bias=eps_t, scale=1.0)
                        nc.scalar.activation(
                            out=rstd, in_=sd,
                            func=mybir.ActivationFunctionType.Exp,
                            scale=-0.5)
                        xc = pp.tile([128, C], BF16, tag="xc", name="xc",
                                     bufs=3)
                        nc.vector.tensor_scalar(
                            out=xc, in0=xcF, scalar1=mv[:, 0:1], scalar2=rstd,
                            op0=mybir.AluOpType.subtract,
                            op1=mybir.AluOpType.mult)
                        pt4 = ps.tile([128, 4, 128], BF16, tag="mm",
                                      name="tp", bufs=2)
                        for cc in range(4):
                            nc.tensor.transpose(
                                pt4[:, cc, :],
                                xc[:, cc * 128:(cc + 1) * 128], ident)
                        nc.vector.tensor_copy(
                            xcT_a[:, :, ct * 128:(ct + 1) * 128], pt4)
                    return go

                def k_chunk(m, hf):
                    def go():
                        pk = ps.tile([128, 512], F32, tag="mm", name="kv",
                                     bufs=2)
                        for cc in range(4):
                            nc.tensor.matmul(
                                pk,
                                wslice(W_K, cc)[:, m * 128:(m + 1) * 128],
                                xcT[cc][:, hf * 512:(hf + 1) * 512],
                                start=(cc == 0),
                                stop=(not use_bias and cc == 3))
                        if use_bias:
                            nc.tensor.matmul(
                                pk, bkrow[:, m * 128:(m + 1) * 128],
                                onesN[:, 0:512], start=False, stop=True)
                        kf = pp.tile([128, 512], BF16, tag="kf",
                                     name="kf", bufs=2)
                        nc.vector.tensor_copy(kf, pk)
                        nc.gpsimd.tensor_copy(
                            kT_[:, m, 0, hf * 512:(hf + 1) * 512], kf)
                        nc.gpsimd.tensor_copy(
                            kT_[:, m, 1, hf * 512:(hf + 1) * 512],
                            kT_[:, m, 0, hf * 512:(hf + 1) * 512])
                    return go

                def v_chunk(ct):
                    def go():
                        pv = ps.tile([128, 512], F32, tag="mm", name="kv",
                                     bufs=2)
                        for cc in range(4):
                            nc.tensor.matmul(
                                pv,
                                xcT[cc][:, ct * 128:(ct + 1) * 128],
                                wslice(W_V, cc),
                                start=(cc == 0),
                                stop=(not use_bias and cc == 3))
                        if use_bias:
                            nc.tensor.matmul(pv, ones1, bvrow,
                                             start=False, stop=True)
                        pvv = pv.rearrange("p (h d) -> p h d", h=H)
                        nc.vector.tensor_copy(va_[:, ct, :, 0:HD], pvv)
                    return go

                def q_chunk(m, ns):
                    def go():
                        pq = ps.tile([128, 512], F32, tag="mm", name="q",
                                     bufs=2)
                        for cc in range(4):
                            nc.tensor.matmul(
                                pq,
                                wslice(W_Q, cc)[:, m * 128:(m + 1) * 128],
                                xTh[:, cc, (ns % 2) * 512:(ns % 2 + 1) * 512],
                                start=(cc == 0), stop=(cc == 3))
                        qf = pp.tile([128, 512], BF16, tag="qf",
                                     name="qf", bufs=2)
                        nc.vector.tensor_copy(qf, pq)
                        nc.gpsimd.tensor_copy(
                            qT_[m][:, 0, ns * 512:(ns + 1) * 512], qf)
                        nc.gpsimd.tensor_sub(
                            qT_[m][:, 1, ns * 512:(ns + 1) * 512], qf,
                            qT_[m][:, 0, ns * 512:(ns + 1) * 512])
                    return go

                chunks = [dma_xtap(0), dma_wtap()]
                chunks += [conv_ct(0), conv_ct(1), dma_xtap(1),
                           conv_ct(2), conv_ct(3), dma_xtap(2)]
                chunks += [k_chunk(m, 0) for m in range(4)]
                chunks += [v_chunk(ct) for ct in range(4)]
                chunks += [conv_ct(4), conv_ct(5), dma_xtap(3),
                           conv_ct(6), conv_ct(7)]
                chunks.append(dma_xth(0))
                chunks += [k_chunk(m, 1) for m in range(4)]
                chunks += [v_chunk(ct) for ct in range(4, 8)]
                chunks += [q_chunk(m, ns) for ns in range(2)
                           for m in range(4)]
                chunks.append(dma_xth(1))
                chunks += [q_chunk(m, ns) for ns in range(2, 4)
                           for m in range(4)]
                return chunks

            # ------------- stage C+D with interleaved chunks -------------
            # cumulative chunk-drain targets after each of the 16 (h,qs)
            # attention slots; the rest drains through the proj phase.
            # chunks drain at score-pair granularity via pair_drain()

            def emit_cd(src, chunks):
                kT_, va_, qT_ = kTz[src], vaug[src], qTb[src]
                emitted = 0

                def drain(target):
                    nonlocal emitted
                    while emitted < min(target, len(chunks)):
                        chunks[emitted]()
                        emitted += 1

                gslot = 0
                nch = len(chunks) if interleave else 0

                def pair_drain():
                    nonlocal gslot
                    gslot += 1
                    if not nch:
                        return
                    if gslot >= 16:
                        # compute chunks: ramp from gslot 16 to 60
                        drain(min(nch, 2 + (gslot - 16) * (nch - 2) // 44))
                    elif gslot >= 1:
                        drain(min(nch, 2))  # the input DMA issues only

                for g in range(2):
                    ots = []
                    for hh in range(4):
                        h = g * 4 + hh
                        m, p0 = h // 2, (h % 2) * 64
                        ot = pp.tile([65, HALF], BF16, tag=f"ot{hh}",
                                     name=f"ot{hh}", bufs=1)
                        ots.append(ot)
                        for qs in range(2):
                            q0 = qs * 1024
                            pav = ps.tile([65, 1024], F32, tag="av",
                                          name="av", bufs=1)
                            for half in range(2):
                                pb = pp.tile([128, 4, 1024], BF16,
                                             tag="probs", name="probs",
                                             bufs=3)
                                for kk in range(4):
                                    kt = half * 4 + kk
                                    pss = ps.tile([128, 1024], F32, tag="s",
                                                  name="s", bufs=2)
                                    for u in range(2):
                                        if use_dr:
                                            nc.tensor.matmul(
                                                pss[:, u * 512:(u + 1) * 512],
                                                kT_[p0:p0 + 64, m, :,
                                                    kt * 128:(kt + 1) * 128],
                                                qT_[m][p0:p0 + 64, :,
                                                       q0 + u * 512:
                                                       q0 + (u + 1) * 512],
                                                start=True, stop=True,
                                                perf_mode=DR)
                                        else:
                                            for pl in range(2):
                                                nc.tensor.matmul(
                                                    pss[:, u * 512:
                                                        (u + 1) * 512],
                                                    kT_[p0:p0 + 64, m, pl,
                                                        kt * 128:
                                                        (kt + 1) * 128],
                                                    qT_[m][p0:p0 + 64, pl,
                                                           q0 + u * 512:
                                                           q0 + (u + 1) * 512],
                                                    start=(pl == 0),
                                                    stop=(pl == 1))
                                    nc.scalar.activation(
                                        out=pb[:, kk, :], in_=pss,
                                        func=mybir.ActivationFunctionType.Exp,
                                        scale=SCALE)
                                    if kt % 2 == 1:
                                        pair_drain()
                                for kk in range(4):
                                    kt = half * 4 + kk
                                    for u in range(2):
                                        nc.tensor.matmul(
                                            pav[:, u * 512:(u + 1) * 512],
                                            va_[:, kt, h, :],
                                            pb[:, kk, u * 512:(u + 1) * 512],
                                            start=(kt == 0), stop=(kt == 7))
                            nc.vector.tensor_copy(ot[:, q0:q0 + 1024], pav)
                    # softmax denominators: pack each head's sum row
                    # [1,2048] into [128,16] (q = p*16+j), take a DVE
                    # reciprocal there, unpack to DRAM, then broadcast.
                    sub_batches = [(0, 4)] if g == 0 else [(0, 2), (2, 2)]
                    for (s0, sn) in sub_batches:
                        packT = pp.tile([128, 4, 16], BF16, tag="packT",
                                        name="packT", bufs=2)
                        for hh in range(s0, s0 + sn):
                            nc.sync.dma_start(
                                out=packT[:, hh - s0, :],
                                in_=ots[hh][64:65, :])
                        packF = pp.tile([128, 4, 16], F32, tag="packF",
                                        name="packF", bufs=2)
                        nc.scalar.activation(
                            out=packF[:, 0:sn, :], in_=packT[:, 0:sn, :],
                            func=mybir.ActivationFunctionType.Ln)
                        packB = pp.tile([128, 4, 16], BF16, tag="packB",
                                        name="packB", bufs=2)
                        nc.scalar.activation(
                            out=packB[:, 0:sn, :], in_=packF[:, 0:sn, :],
                            func=mybir.ActivationFunctionType.Exp,
                            scale=-1.0)
                        dr = drp.tile([4, HALF], BF16, tag="dr", name="dr")
                        for hh in range(s0, s0 + sn):
                            nc.sync.dma_start(
                                out=dr[hh - s0:hh - s0 + 1, :],
                                in_=packB[:, hh - s0, :])
                        for hh in range(s0, s0 + sn):
                            h = g * 4 + hh
                            m, p0 = h // 2, (h % 2) * 64
                            rb = pp.tile([64, HALF], BF16, tag="rb",
                                         name="rb", bufs=2)
                            nc.sync.dma_start(
                                out=rb,
                                in_=dr[hh - s0:hh - s0 + 1,
                                       :].partition_broadcast(64))
                            nc.vector.tensor_mul(
                                outA[m][p0:p0 + 64, :], ots[hh][0:64, :],
                                rb)

                # ---------------- stage D: proj ----------------
                n_proj = 16
                rem = len(chunks) - emitted
                for pi, (m, ns) in enumerate(
                        (m, ns) for m in range(4) for ns in range(4)):
                    pd = ps.tile([128, 512], F32, tag="mm", name="p",
                                 bufs=2)
                    for cc in range(4):
                        nc.tensor.matmul(
                            pd,
                            wslice(W_P, cc)[:, m * 128:(m + 1) * 128],
                            outA[cc][:, ns * 512:(ns + 1) * 512],
                            start=(cc == 0), stop=(cc == 3))
                    ys = pp.tile([128, 512], F32, tag="y", name="y",
                                 bufs=3)
                    nc.vector.tensor_scalar(
                        out=ys, in0=pd, scalar1=bp[m], scalar2=None,
                        op0=mybir.AluOpType.add)
                    nc.sync.dma_start(
                        out=yt_e[m * 128:(m + 1) * 128,
                                 ns * 512:(ns + 1) * 512], in_=ys)
                    drain((64 + pi + 1) * len(chunks) // 72
                          if rem > 0 else 0)
                drain(len(chunks))

            # prologue: fill the pipeline for buffer 0
            for ch in ab_chunks(0):
                ch()

            if loop_reps:
                assert loop_reps % 2 == 0
                with tc.For_i(0, loop_reps // 2, 1):
                    if parts == "cd":
                        emit_cd(0, [])
                        emit_cd(0, [])
                    elif parts == "ab":
                        for ch in ab_chunks(1):
                            ch()
                        for ch in ab_chunks(0):
                            ch()
                    else:
                        emit_cd(0, ab_chunks(1))
                        emit_cd(1, ab_chunks(0))
            else:
                for i in range(reps):
                    nxt = ab_chunks((i + 1) % 2) if i + 1 < reps else []
                    emit_cd(i % 2, nxt)

    if fixup:
        _fixup_sync_waits(nc)
    return nc


_NC = {}


def _get_nc(use_bias=True):
    if use_bias not in _NC:
        _NC[use_bias] = build_nc(use_bias=use_bias)
    return _NC[use_bias]


def _host_prep(x, Wq, Wkv, sr_w, sr_b, ln_g, ln_b, proj_w, proj_b):
    bf = ml_dtypes.bfloat16
    f32 = np.float32
    x = np.asarray(x, f32)
    Wq = np.asarray(Wq, f32)
    Wkv = np.asarray(Wkv, f32)
    sr_w = np.asarray(sr_w, f32)
    sr_b = np.asarray(sr_b, f32)
    ln_g = np.asarray(ln_g, f32)
    ln_b = np.asarray(ln_b, f32)
    proj_w = np.asarray(proj_w, f32)
    proj_b = np.asarray(proj_b, f32)

    Wk, Wv = Wkv[:C], Wkv[C:]
    wtap = np.concatenate(
        [sr_w[:, :, 0, 0].T, sr_w[:, :, 0, 1].T,
         sr_w[:, :, 1, 0].T, sr_w[:, :, 1, 1].T], axis=0)
    # Wq is NOT pre-scaled: the softmax exp applies SCALE on the logits.
    wrest = np.concatenate(
        [(Wk * ln_g[None, :]).T, (Wv * ln_g[None, :]).T,
         Wq.T, proj_w.T], axis=0)
    weights = {
        "wtap": np.ascontiguousarray(wtap).astype(bf),
        "wrest": np.ascontiguousarray(wrest).astype(bf),
        "srb": sr_b[None, :].astype(bf),
        "bkrow": (Wk @ ln_b)[None, :].astype(bf),
        "bvrow": (Wv @ ln_b)[None, :].astype(bf),
        "bp": proj_b[:, None].astype(f32),
        "ident": np.eye(128, dtype=f32).astype(bf),
    }

    hh, ww = np.meshgrid(np.arange(RS), np.arange(RS), indexing="ij")
    tap_idx = [((2 * hh + kh) * SIDE + 2 * ww + kw).ravel()
               for kh in range(2) for kw in range(2)]

    in_maps = []
    xtap_cache = {}
    for core in range(8):
        b, half = core // 2, core % 2
        m = dict(weights)
        if b not in xtap_cache:
            xT = np.ascontiguousarray(x[b].T).astype(bf)     # [C, N]
            xtap_cache[b] = (
                xT, np.concatenate([xT[:, ti] for ti in tap_idx], axis=0))
        xT, xtap = xtap_cache[b]
        m["xtap"] = xtap
        m["xth"] = np.ascontiguousarray(
            xT[:, half * HALF:(half + 1) * HALF])
        in_maps.append(m)
    return in_maps


def kernel(**inputs):
    in_maps = _host_prep(**inputs)
    use_bias = bool(np.any(np.asarray(inputs["sr_b"]))
                    or np.any(np.asarray(inputs["ln_b"]))
                    or np.any(np.asarray(inputs["proj_b"])))
    nc = _get_nc(use_bias)
    res = run_bass_kernel_spmd(nc, in_maps, core_ids=list(range(8)))
    out = np.empty((B, N, C), np.float32)
    for core in range(8):
        b, half = core // 2, core % 2
        out[b, half * HALF:(half + 1) * HALF, :] = res.results[core]["yT"].T
    return out

